# revision 1
# baseline (speedup 1.0000x reference)
"""BiLSTM (B=256, T=512, I=128, H=256) Trainium2 kernel.

Strategy: data-parallel over batch x direction across 8 NeuronCores.
Each core runs one LSTM direction over a 64-row batch slice (cores 0-3
forward, cores 4-7 backward on host-time-reversed x), fully on-chip:
x resident in SBUF (pre-transposed on host to [I, b, t]), gates built in
PSUM by f32r matmuls (bias via K=1 ones-matmul, x-projection, two
recurrent K-chunks), single-sigmoid activation for all 4 gates (the
g-gate tanh is computed as 2*sigmoid(2x)-1 with the 2x folded into the
host-prepared weights), elementwise c/h update on DVE/GpSimd, and the
next-step h^T produced by PE transposes.  Each core also computes its
partial fc projection h_T @ W_fc_half^T; the host sums the two partials
per batch row and adds b_fc.
"""

import sys

sys.path.insert(0, "/opt/trn_rl_repo")

import numpy as np

import concourse.mybir as mybir
import concourse.tile as tile
from concourse import bacc
from concourse.masks import make_identity

F32 = mybir.dt.float32
F32R = mybir.dt.float32r
B = 64  # batch rows per core
I = 128
H = 256
G4 = 4 * H
T = 512

# torch gate order (i,f,g,o) -> kernel order (i,f,o,g)
_PERM = np.r_[0:256, 256:512, 768:1024, 512:768]


def _build_nc():
    nc = bacc.Bacc("TRN2", debug=False, num_devices=8)
    xT_d = nc.dram_tensor("xT", [I, B, T], F32R, kind="ExternalInput")
    wihT_d = nc.dram_tensor("wihT", [I, G4], F32R, kind="ExternalInput")
    whhT_d = nc.dram_tensor("whhT", [H, G4], F32R, kind="ExternalInput")
    bias_d = nc.dram_tensor("bias", [1, G4], F32R, kind="ExternalInput")
    wfcT_d = nc.dram_tensor("wfcT", [H, 1], F32, kind="ExternalInput")
    fc_d = nc.dram_tensor("fc_part", [B, 1], F32, kind="ExternalOutput")
    h_d = nc.dram_tensor("h_out", [B, H], F32, kind="ExternalOutput")

    with tile.TileContext(nc) as tc:
        _body(tc, xT_d, wihT_d, whhT_d, bias_d, wfcT_d, fc_d, h_d)
    nc.compile()
    return nc


def _body(tc, xT_d, wihT_d, whhT_d, bias_d, wfcT_d, fc_d, h_d):
    from contextlib import ExitStack

    nc = tc.nc
    ctx = ExitStack()
    singles = ctx.enter_context(tc.tile_pool(name="singles", bufs=1))
    state = ctx.enter_context(tc.tile_pool(name="state", bufs=3))
    acts = ctx.enter_context(tc.tile_pool(name="acts", bufs=3))
    psum_g = ctx.enter_context(tc.tile_pool(name="psum_g", bufs=3, space="PSUM"))
    psum_t = ctx.enter_context(tc.tile_pool(name="psum_t", bufs=2, space="PSUM"))

    xT = singles.tile([I, B, T], F32R)
    nc.sync.dma_start(out=xT, in_=xT_d.ap())
    wihT = singles.tile([I, G4], F32R)
    nc.sync.dma_start(out=wihT, in_=wihT_d.ap())
    whhT0 = singles.tile([128, G4], F32R)
    whhT1 = singles.tile([128, G4], F32R)
    nc.sync.dma_start(out=whhT0, in_=whhT_d.ap()[0:128, :])
    nc.sync.dma_start(out=whhT1, in_=whhT_d.ap()[128:256, :])
    whhT = [whhT0, whhT1]
    bias_sb = singles.tile([1, G4], F32R)
    nc.sync.dma_start(out=bias_sb, in_=bias_d.ap())
    wfcT0 = singles.tile([128, 1], F32)
    wfcT1 = singles.tile([128, 1], F32)
    nc.sync.dma_start(out=wfcT0, in_=wfcT_d.ap()[0:128, :])
    nc.sync.dma_start(out=wfcT1, in_=wfcT_d.ap()[128:256, :])
    ones_sb = singles.tile([1, 128], F32R)
    nc.vector.memset(ones_sb.bitcast(F32), 1.0)
    ident = singles.tile([64, 64], F32)
    make_identity(nc, ident)

    hT = state.tile([128, 2, B], F32R)  # hT[:, kc, b] = h[b, 128*kc + p]
    nc.vector.memset(hT.bitcast(F32), 0.0)
    c = state.tile([B, H], F32)
    nc.vector.memset(c, 0.0)

    h_sb = None
    for t in range(T):
        gp = psum_g.tile([B, G4], F32)
        xt = xT[:, :, t : t + 1]  # [128, 64, 1] -> lhsT M=64
        for n in range(2):
            ncol = slice(512 * n, 512 * n + 512)
            nc.tensor.matmul(
                gp[:, ncol], ones_sb[:, 0:64], bias_sb[:, ncol],
                start=True, stop=False,
            )
            nc.tensor.matmul(
                gp[:, ncol], xt, wihT[:, ncol], start=False, stop=False
            )
            for kc in range(2):
                nc.tensor.matmul(
                    gp[:, ncol], hT[:, kc, :], whhT[kc][:, ncol],
                    start=False, stop=(kc == 1),
                )
        # all gates through one sigmoid; g-gate = tanh via 2*sig(2x)-1
        sig = acts.tile([B, G4], F32)
        nc.scalar.activation(sig, gp, mybir.ActivationFunctionType.Sigmoid)
        # c' = f*c + i*(2*sg - 1) = (f*c) + (2*(i*sg) - i)
        u = acts.tile([B, H], F32)
        nc.vector.tensor_mul(u, sig[:, 0:256], sig[:, 768:1024])
        w = acts.tile([B, H], F32)
        nc.vector.scalar_tensor_tensor(
            out=w, in0=u, scalar=2.0, in1=sig[:, 0:256],
            op0=mybir.AluOpType.mult, op1=mybir.AluOpType.subtract,
        )
        fc_ = acts.tile([B, H], F32)
        nc.gpsimd.tensor_mul(fc_, sig[:, 256:512], c)
        c = state.tile([B, H], F32)
        nc.vector.tensor_add(c, fc_, w)
        th = acts.tile([B, H], F32)
        nc.scalar.activation(th, c, mybir.ActivationFunctionType.Tanh)
        h_sb = acts.tile([B, H], F32)
        nc.gpsimd.tensor_mul(h_sb, sig[:, 512:768], th)
        hT_ps = psum_t.tile([128, 2, B], F32)
        for kc in range(2):
            nc.tensor.transpose(
                hT_ps[:, kc, :], h_sb[:, 128 * kc : 128 * kc + 128], ident
            )
        hT = state.tile([128, 2, B], F32R)
        nc.vector.tensor_copy(hT, hT_ps)

    fc_ps = psum_t.tile([B, 1], F32, tag="hT_ps")
    nc.tensor.matmul(
        fc_ps, hT[:, 0, :].bitcast(F32), wfcT0, start=True, stop=False
    )
    nc.tensor.matmul(
        fc_ps, hT[:, 1, :].bitcast(F32), wfcT1, start=False, stop=True
    )
    fc_sb = acts.tile([B, 1], F32)
    nc.vector.tensor_copy(fc_sb, fc_ps)
    nc.sync.dma_start(out=fc_d.ap(), in_=fc_sb)
    nc.sync.dma_start(out=h_d.ap(), in_=h_sb)
    ctx.close()


def _prep_weights(Wih, Whh, bih, bhh, wfc_half):
    wihT = np.array(np.asarray(Wih, np.float32).T[:, _PERM], dtype=np.float32)
    whhT = np.array(np.asarray(Whh, np.float32).T[:, _PERM], dtype=np.float32)
    bias = np.array(
        (np.asarray(bih, np.float32) + np.asarray(bhh, np.float32))[_PERM][None, :],
        dtype=np.float32,
    )
    for a in (wihT, whhT, bias):
        a[:, 768:1024] *= 2.0  # g-gate tanh-via-sigmoid prescale
    return {
        "wihT": np.ascontiguousarray(wihT),
        "whhT": np.ascontiguousarray(whhT),
        "bias": np.ascontiguousarray(bias),
        "wfcT": np.ascontiguousarray(
            np.asarray(wfc_half, np.float32).reshape(-1, 1)
        ),
    }


_NC_CACHE = {}


def _get_nc():
    if "nc" not in _NC_CACHE:
        _NC_CACHE["nc"] = _build_nc()
    return _NC_CACHE["nc"]


def make_in_maps(x, Wih_f, Whh_f, bih_f, bhh_f, Wih_b, Whh_b, bih_b, bhh_b, W_fc):
    W_fc = np.asarray(W_fc, np.float32)
    w_f = _prep_weights(Wih_f, Whh_f, bih_f, bhh_f, W_fc[0, 0:256])
    w_b = _prep_weights(Wih_b, Whh_b, bih_b, bhh_b, W_fc[0, 256:512])
    x = np.asarray(x, np.float32)
    in_maps = []
    for core in range(8):
        fwd = core < 4
        chunk = core % 4
        xc = x[64 * chunk : 64 * chunk + 64]
        if not fwd:
            xc = xc[:, ::-1, :]
        m = dict(w_f if fwd else w_b)
        m["xT"] = np.ascontiguousarray(np.transpose(xc, (2, 0, 1)))
        in_maps.append(m)
    return in_maps


def gather_output(results, b_fc):
    out = np.zeros((256, 1), dtype=np.float32)
    for core in range(8):
        chunk = core % 4
        out[64 * chunk : 64 * chunk + 64] += results[core]["fc_part"]
    return out + np.asarray(b_fc, np.float32)


def kernel(x, Wih_f, Whh_f, bih_f, bhh_f, Wih_b, Whh_b, bih_b, bhh_b, W_fc, b_fc):
    from concourse.bass_utils import run_bass_kernel_spmd

    nc = _get_nc()
    in_maps = make_in_maps(
        x, Wih_f, Whh_f, bih_f, bhh_f, Wih_b, Whh_b, bih_b, bhh_b, W_fc
    )
    res = run_bass_kernel_spmd(nc, in_maps, core_ids=list(range(8)))
    return gather_output(res.results, b_fc)
